# revision 12
# baseline (speedup 1.0000x reference)
"""Trainium2 Bass kernel for nn_CustomLoraLinear (GPTQ 4-bit + ternary LoRA).

Strategy (tensor-parallel over out_features, 8 cores, 512 cols each):

  reference:  AB = A^T B^T;  markers m from (AB, w);  AB2 = AB - 1.5*m
              grouped = group_mean(AB2); zero_plus = grouped[g]/1.5
              W = scales[g] * (w - zeros[g] + m + zero_plus);  out = x @ W

  kernel:     W  = W1 + G @ T2     (G = one-hot of g_idx, rank-32 structure)
              W1 = scales[g] * (w + m)              (per-element, per k-tile)
              T2 = scales * (grouped/1.5 - zeros)   ([32, 512] table)
              out = x @ W1 + (x @ G) @ T2
  The rank-32 factoring removes the global barrier (group means need ALL
  rows) from the main-matmul critical path.  group_mean(AB) is computed
  analytically through the LoRA factors (A_grp = G^T A^T is exact small
  ints), so only the markers' group-sums need the elementwise pipeline.

  The IN axis is permuted (i' = j*512 + p; qweight row p packs nibble j)
  so each 128-row k-tile unpacks with one uniform shift; the contraction
  is permutation-invariant so only host-side layout changes.

  All math on device; host does layout only (shard/transpose/permute/
  bf16-cast of inputs, index->indicator/iota constants).
"""
import numpy as np
import ml_dtypes

import concourse.bass as bass
import concourse.mybir as mybir
import concourse.tile as tile
from concourse import bacc
from concourse.bass_utils import run_bass_kernel_spmd

BF16 = ml_dtypes.bfloat16
F32 = mybir.dt.float32
BF = mybir.dt.bfloat16
I32 = mybir.dt.int32
AOP = mybir.AluOpType
PSUM = bass.MemorySpace.PSUM

IN, OUT, R, GS, NG, PF, MAXQ = 4096, 4096, 16, 128, 32, 8, 15
NCORES = 8
SH = OUT // NCORES          # 512 out columns per core
KT = IN // 128              # 32 k-tiles
QB = IN // PF // 128        # 4 q row-blocks of 128


def build_bass(m_tot: int):
    """Build the per-core Bass program (same program on all 8 cores)."""
    MC = m_tot // 512       # m-chunks of 512 rows
    nc = bacc.Bacc("TRN2", target_bir_lowering=False, debug=False)

    xT = nc.declare_dram_parameter("xT", [IN, m_tot], BF, isOutput=False)
    A = nc.declare_dram_parameter("A", [R, IN], BF, isOutput=False)
    AT = nc.declare_dram_parameter("AT", [IN, R], BF, isOutput=False)
    BTd = nc.declare_dram_parameter("BT", [R, SH], BF, isOutput=False)
    qd = nc.declare_dram_parameter("q", [IN // PF, SH], I32, isOutput=False)
    scd = nc.declare_dram_parameter("sc", [NG, SH], F32, isOutput=False)
    qzd = nc.declare_dram_parameter("qz", [NG, SH // PF], I32, isOutput=False)
    gftd = nc.declare_dram_parameter("gft", [128, KT], F32, isOutput=False)
    iotad = nc.declare_dram_parameter("iota", [128, NG], F32, isOutput=False)
    c32d = nc.declare_dram_parameter("c32", [NG, 1], F32, isOutput=False)
    gbd = nc.declare_dram_parameter("gb", [NG, IN], F32, isOutput=False)
    sum4d = nc.declare_dram_parameter("sum4", [128, NG], BF, isOutput=False)
    outd = nc.declare_dram_parameter("out", [m_tot, SH], F32, isOutput=True)

    xT_t = xT.ap().rearrange("(t p) m -> p t m", p=128)

    with tile.TileContext(nc) as tc:
        with (
            tc.tile_pool(name="const", bufs=1) as cpool,
            tc.tile_pool(name="wsb", bufs=KT) as wpool,
            tc.tile_pool(name="xg", bufs=2) as xgpool,
            tc.tile_pool(name="prep", bufs=2) as ppool,
            tc.tile_pool(name="prepbf", bufs=2) as pbfpool,
            tc.tile_pool(name="xgt", bufs=2) as xgtpool,
            tc.tile_pool(name="ostage", bufs=3) as opool,
            tc.tile_pool(name="pmain", bufs=4, space=PSUM) as pmain,
        ):
            # ---------------- constants / setup ----------------
            A_sb = cpool.tile([R, IN], BF)
            nc.sync.dma_start(A_sb[:], A[:])
            AT_sb = cpool.tile([128, KT, R], BF)
            nc.sync.dma_start(AT_sb[:], AT.ap().rearrange("(t p) r -> p t r", p=128))
            BT_sb = cpool.tile([R, SH], BF)
            nc.sync.dma_start(BT_sb[:], BTd[:])
            q_sb = cpool.tile([128, QB, SH], I32)
            nc.sync.dma_start(q_sb[:], qd.ap().rearrange("(b p) o -> p b o", p=128))
            sc_sb = cpool.tile([NG, SH], F32)
            nc.sync.dma_start(sc_sb[:], scd[:])
            qz_sb = cpool.tile([NG, SH // PF], I32)
            nc.sync.dma_start(qz_sb[:], qzd[:])
            gft_sb = cpool.tile([128, KT], F32)
            nc.sync.dma_start(gft_sb[:], gftd[:])
            iota_sb = cpool.tile([128, NG], F32)
            nc.sync.dma_start(iota_sb[:], iotad[:])
            c32_sb = cpool.tile([NG, 1], F32)
            nc.sync.dma_start(c32_sb[:], c32d[:])
            gb_sb = cpool.tile([NG, IN], F32)
            nc.sync.dma_start(gb_sb[:], gbd[:])
            sum4_sb = cpool.tile([128, NG], BF)
            nc.sync.dma_start(sum4_sb[:], sum4d[:])

            # scales in bf16 (rhs of the T1 gather matmul)
            sc_bf = cpool.tile([NG, SH], BF)
            nc.vector.tensor_copy(sc_bf[:], sc_sb[:])
            sc192 = cpool.tile([NG, SH], F32)
            nc.vector.tensor_scalar(sc192[:], sc_sb[:], 1.0 / 192.0, None,
                                    AOP.mult)

            # zeros unpack: [NG, SH//PF] int32 -> [NG, SH] (int, then cast)
            zeros_i = cpool.tile([NG, SH], I32)
            for j in range(PF):
                nc.vector.tensor_scalar(
                    zeros_i[:, j::PF], qz_sb[:], 4 * j, MAXQ,
                    AOP.logical_shift_right, AOP.bitwise_and)
            zeros_f = cpool.tile([NG, SH], F32)
            nc.vector.tensor_copy(zeros_f[:], zeros_i[:])

            # one-hot tiles: G[p, t, g] = (g_idx[128t+p] == g)   (bf16)
            G_sb = cpool.tile([128, KT, NG], BF)
            for t in range(KT):
                nc.vector.tensor_scalar(
                    G_sb[:, t, :], iota_sb[:], gft_sb[:, t:t + 1], None,
                    AOP.is_equal)
            # transposed one-hot: GT[g, t, i] = (g_idx[128t+i] == g)
            GT_sb = cpool.tile([NG, KT, 128], BF)
            for t in range(KT):
                nc.gpsimd.tensor_scalar(
                    GT_sb[:, t, :], gb_sb[:, 128 * t:128 * (t + 1)],
                    c32_sb[:], None, AOP.is_equal)

            # A_grpT[r, g] = sum_{i in group g} A^T[i, r]   (exact ints)
            with tc.tile_pool(name="pmisc", bufs=1, space=PSUM) as pmisc:
                pAg = pmisc.tile([R, NG], F32)
                for t in range(KT):
                    nc.tensor.matmul(pAg[:], AT_sb[:, t, :], G_sb[:, t, :],
                                     start=(t == 0), stop=(t == KT - 1))
                AgT_sb = cpool.tile([R, NG], BF)
                nc.vector.tensor_copy(AgT_sb[:], pAg[:])
                # sumAB_g[g, o] = sum_{i in g} AB[i, o]  (exact ints)
                psAB = pmisc.tile([NG, SH], F32)
                nc.tensor.matmul(psAB[:], AgT_sb[:], BT_sb[:],
                                 start=True, stop=True)
                sAB_sb = cpool.tile([NG, SH], F32)
                nc.vector.tensor_copy(sAB_sb[:], psAB[:])

            # x chunk for mc 0 (consumed by matmuls interleaved into prep)
            xg0 = xgpool.tile([128, KT, 512], BF, tag="xg")
            nc.sync.dma_start(xg0[:], xT_t[:, :, 0:512])

            pm0 = [pmain.tile([128, 512], F32, tag="pm", name=f"pm0_{i}")
                   for i in range(4)]

            # ---------------- prep loop over 32 k-tiles ----------------
            W_sb = []
            with (
                tc.tile_pool(name="pmg", bufs=1, space=PSUM) as pmgpool,
                tc.tile_pool(name="pab", bufs=1, space=PSUM) as pabpool,
                tc.tile_pool(name="pt1", bufs=1, space=PSUM) as pt1pool,
            ):
                pmg = pmgpool.tile([NG, SH], F32)   # group-sums of markers
                for t in range(KT):
                    pab = pabpool.tile([128, SH], F32)
                    nc.tensor.matmul(pab[:], A_sb[:, 128 * t:128 * (t + 1)],
                                     BT_sb[:], start=True, stop=True)
                    # w = (q >> 4j) & 15  (int32), then cast to f32
                    wi = ppool.tile([128, SH], I32, tag="wi")
                    nc.vector.tensor_scalar(wi[:], q_sb[:, t % QB, :],
                                            4 * (t // QB), MAXQ,
                                            AOP.logical_shift_right,
                                            AOP.bitwise_and)
                    wf = ppool.tile([128, SH], F32, tag="wf")
                    nc.gpsimd.tensor_copy(wf[:], wi[:])
                    wne15 = ppool.tile([128, SH], F32, tag="wne15")
                    nc.gpsimd.tensor_scalar(wne15[:], wf[:], float(MAXQ), None,
                                            AOP.is_lt)
                    wne0 = ppool.tile([128, SH], F32, tag="wne0")
                    nc.gpsimd.tensor_scalar(wne0[:], wf[:], 0.0, None,
                                            AOP.is_gt)
                    pp = ppool.tile([128, SH], F32, tag="pp")
                    nc.vector.scalar_tensor_tensor(pp[:], pab[:], 2.0,
                                                   wne15[:], AOP.is_ge,
                                                   AOP.mult)
                    nn = ppool.tile([128, SH], F32, tag="nn")
                    nc.vector.scalar_tensor_tensor(nn[:], pab[:], -2.0,
                                                   wne0[:], AOP.is_le,
                                                   AOP.mult)
                    mf = ppool.tile([128, SH], F32, tag="mf")
                    nc.vector.tensor_sub(mf[:], pp[:], nn[:])
                    mbf = pbfpool.tile([128, SH], BF, tag="mbf")
                    nc.scalar.copy(mbf[:], mf[:])
                    # group-sums of markers (accumulated over all k-tiles)
                    nc.tensor.matmul(pmg[:], G_sb[:, t, :], mbf[:],
                                     start=(t == 0), stop=(t == KT - 1))
                    # T1 gather: scales[g_idx[i], o]
                    pt1 = pt1pool.tile([128, SH], F32)
                    nc.tensor.matmul(pt1[:], GT_sb[:, t, :], sc_bf[:],
                                     start=True, stop=True)
                    wpm = ppool.tile([128, SH], F32, tag="wpm")
                    nc.vector.tensor_add(wpm[:], wf[:], mf[:])
                    wt = wpool.tile([128, SH], BF, tag="w")
                    nc.vector.tensor_mul(wt[:], pt1[:], wpm[:])
                    W_sb.append(wt)
                    # keep PE busy: main matmuls for m-chunk 0, k-tile t
                    for mt in range(4):
                        nc.tensor.matmul(
                            pm0[mt][:], xg0[:, t, 128 * mt:128 * (mt + 1)],
                            wt[:], start=(t == 0), stop=False)

                # ------------- T2 table -------------
                # sumAB2_g = sumAB_g - 1.5*sum_markers_g
                # T2 = sc/192*sumAB2_g - sc*zeros
                g1 = ppool.tile([NG, SH], F32, tag="pp")
                nc.vector.scalar_tensor_tensor(g1[:], pmg[:], -1.5, sAB_sb[:],
                                               AOP.mult, AOP.add)
                d1 = ppool.tile([NG, SH], F32, tag="nn")
                nc.vector.scalar_tensor_tensor(d1[:], zeros_f[:], -192.0,
                                               g1[:], AOP.mult, AOP.add)
                T2_sb = cpool.tile([NG, SH], BF)
                nc.vector.tensor_mul(T2_sb[:], d1[:], sc192[:])

            # ---------------- main loop over m-chunks ----------------
            with tc.tile_pool(name="pxg", bufs=4, space=PSUM) as pxgpool:
                for mc in range(MC):
                    if mc == 0:
                        xg = xg0
                    else:
                        xg = xgpool.tile([128, KT, 512], BF, tag="xg", name="xg")
                        nc.sync.dma_start(
                            xg[:], xT_t[:, :, 512 * mc:512 * (mc + 1)])
                    # xG^T chunk: col-packed one-hot matmuls (4 concurrent
                    # in the PE array via tile_position; one bank per group)
                    pxg = [pxgpool.tile([128, 512], F32, tag="pxg",
                                        name=f"pxg{c}") for c in range(4)]
                    for t in range(KT):
                        c = t % 4
                        nc.tensor.matmul(
                            pxg[c][32 * c:32 * (c + 1), :], G_sb[:, t, :],
                            xg[:, t, :], start=(t < 4), stop=(t >= KT - 4),
                            tile_position=(0, 32 * c))
                    xg4 = xgtpool.tile([128, 512], BF, tag="xg4")
                    for c in range(4):
                        nc.vector.tensor_copy(xg4[32 * c:32 * (c + 1), :],
                                              pxg[c][32 * c:32 * (c + 1), :])
                    pfix = pxgpool.tile([NG, 512], F32, tag="pxg", name="pfix")
                    nc.tensor.matmul(pfix[:], sum4_sb[:], xg4[:],
                                     start=True, stop=True)
                    xgt = xgtpool.tile([NG, 512], BF, tag="xgt")
                    nc.vector.tensor_copy(xgt[:], pfix[:])

                    for mt in range(4):
                        if mc == 0:
                            pm = pm0[mt]
                            nc.tensor.matmul(pm[:],
                                             xgt[:, 128 * mt:128 * (mt + 1)],
                                             T2_sb[:], start=False, stop=True)
                        else:
                            pm = pmain.tile([128, 512], F32, tag="pm", name="pm")
                            nc.tensor.matmul(pm[:],
                                             xgt[:, 128 * mt:128 * (mt + 1)],
                                             T2_sb[:], start=True, stop=False)
                            for t in range(KT):
                                nc.tensor.matmul(
                                    pm[:], xg[:, t, 128 * mt:128 * (mt + 1)],
                                    W_sb[t][:], start=False,
                                    stop=(t == KT - 1))
                        ost = opool.tile([128, SH], F32)
                        nc.vector.tensor_copy(ost[:], pm[:])
                        nc.sync.dma_start(
                            outd[512 * mc + 128 * mt:
                                 512 * mc + 128 * (mt + 1), :], ost[:])
    nc.compile()
    return nc


def host_prep(x, lora_A, lora_B, qweight, qzeros, scales, g_idx):
    """Layout-only host prep: shard, permute IN axis, transpose, bf16-cast."""
    m_tot = int(np.prod(x.shape[:-1]))
    orig = (np.arange(IN) % (IN // PF)) * PF + np.arange(IN) // (IN // PF)

    X2 = np.asarray(x, np.float32).reshape(m_tot, IN)
    xT = np.ascontiguousarray(X2.T[orig]).astype(BF16)          # [IN, m_tot]
    Ap = np.ascontiguousarray(
        np.asarray(lora_A, np.float32)[:, orig]).astype(BF16)   # [R, IN]
    ATp = np.ascontiguousarray(Ap.T)                            # [IN, R]
    gperm = np.asarray(g_idx)[orig].astype(np.float32)          # [IN]
    gft = np.ascontiguousarray(gperm.reshape(KT, 128).T)        # [128, KT]
    gb = np.ascontiguousarray(np.broadcast_to(gperm, (NG, IN)))
    iota = np.ascontiguousarray(np.broadcast_to(
        np.arange(NG, dtype=np.float32), (128, NG)))
    c32 = np.arange(NG, dtype=np.float32).reshape(NG, 1)
    sum4 = (np.arange(128)[:, None] % NG
            == np.arange(NG)[None, :]).astype(BF16)

    in_maps = []
    for r in range(NCORES):
        cs = slice(SH * r, SH * (r + 1))
        zs = slice((SH // PF) * r, (SH // PF) * (r + 1))
        in_maps.append(dict(
            xT=xT, A=Ap, AT=ATp,
            BT=np.ascontiguousarray(
                np.asarray(lora_B, np.float32)[cs, :].T).astype(BF16),
            q=np.ascontiguousarray(np.asarray(qweight)[:, cs]),
            sc=np.ascontiguousarray(np.asarray(scales, np.float32)[:, cs]),
            qz=np.ascontiguousarray(np.asarray(qzeros)[:, zs]),
            gft=gft, iota=iota, c32=c32, gb=gb, sum4=sum4,
        ))
    return in_maps, m_tot


LAST_RESULT = None


def kernel(**inputs):
    global LAST_RESULT
    x = np.asarray(inputs["x"])
    in_maps, m_tot = host_prep(
        x, inputs["lora_A"], inputs["lora_B"], inputs["qweight"],
        inputs["qzeros"], inputs["scales"], inputs["g_idx"])
    nc = build_bass(m_tot)
    res = run_bass_kernel_spmd(nc, in_maps, core_ids=list(range(NCORES)))
    LAST_RESULT = res
    outs = [np.asarray(res.results[r]["out"], np.float32)
            for r in range(NCORES)]
    out = np.concatenate(outs, axis=1)
    return out.reshape(*x.shape[:-1], OUT)


# revision 14
# speedup vs baseline: 120.7888x; 120.7888x over previous
"""Trainium2 Bass kernel for nn_CustomLoraLinear (GPTQ 4-bit + ternary LoRA).

Strategy (tensor-parallel over out_features, 8 cores, 512 cols each):

  reference:  AB = A^T B^T;  markers m from (AB, w);  AB2 = AB - 1.5*m
              grouped = group_mean(AB2); zero_plus = grouped[g]/1.5
              W = scales[g] * (w - zeros[g] + m + zero_plus);  out = x @ W

  kernel:     W  = W1 + G @ T2     (G = one-hot of g_idx, rank-32 structure)
              W1 = scales[g] * (w + m)              (per-element, per k-tile)
              T2 = scales * (grouped/1.5 - zeros)   ([32, 512] table)
              out = x @ W1 + (x @ G) @ T2
  The rank-32 factoring removes the global barrier (group means need ALL
  rows) from the main-matmul critical path.  group_mean(AB) is computed
  analytically through the LoRA factors (A_grp = G^T A^T is exact small
  ints), so only the markers' group-sums need the elementwise pipeline.

  The IN axis is permuted (i' = j*512 + p; qweight row p packs nibble j)
  so each 128-row k-tile unpacks with one uniform shift; the contraction
  is permutation-invariant so only host-side layout changes.

  All math on device; host does layout only (shard/transpose/permute/
  bf16-cast of inputs, index->indicator/iota constants).
"""
import numpy as np
import ml_dtypes

import concourse.bass as bass
import concourse.mybir as mybir
import concourse.tile as tile
from concourse import bacc
from concourse.bass_utils import run_bass_kernel_spmd

BF16 = ml_dtypes.bfloat16
F32 = mybir.dt.float32
BF = mybir.dt.bfloat16
I32 = mybir.dt.int32
AOP = mybir.AluOpType
PSUM = bass.MemorySpace.PSUM

IN, OUT, R, GS, NG, PF, MAXQ = 4096, 4096, 16, 128, 32, 8, 15
NCORES = 8
SH = OUT // NCORES          # 512 out columns per core
KT = IN // 128              # 32 k-tiles
QB = IN // PF // 128        # 4 q row-blocks of 128


def build_bass(m_tot: int, mode: str = "full"):
    """Build the per-core Bass program (same program on all 8 cores)."""
    MC = m_tot // 512       # m-chunks of 512 rows
    nc = bacc.Bacc("TRN2", target_bir_lowering=False, debug=False)

    xT = nc.declare_dram_parameter("xT", [IN, m_tot], BF, isOutput=False)
    A = nc.declare_dram_parameter("A", [R, IN], BF, isOutput=False)
    AT = nc.declare_dram_parameter("AT", [IN, R], BF, isOutput=False)
    BTd = nc.declare_dram_parameter("BT", [R, SH], BF, isOutput=False)
    qd = nc.declare_dram_parameter("q", [IN // PF, SH], I32, isOutput=False)
    scd = nc.declare_dram_parameter("sc", [NG, SH], F32, isOutput=False)
    qzd = nc.declare_dram_parameter("qz", [NG, SH // PF], I32, isOutput=False)
    gftd = nc.declare_dram_parameter("gft", [128, KT], F32, isOutput=False)
    iotad = nc.declare_dram_parameter("iota", [128, NG], F32, isOutput=False)
    c32d = nc.declare_dram_parameter("c32", [NG, 1], F32, isOutput=False)
    gbd = nc.declare_dram_parameter("gb", [NG, IN], F32, isOutput=False)
    sum4d = nc.declare_dram_parameter("sum4", [128, NG], BF, isOutput=False)
    outd = nc.declare_dram_parameter("out", [m_tot, SH], F32, isOutput=True)

    xT_t = xT.ap().rearrange("(t p) m -> p t m", p=128)

    with tile.TileContext(nc) as tc:
        with (
            tc.tile_pool(name="const", bufs=1) as cpool,
            tc.tile_pool(name="wsb", bufs=KT) as wpool,
            tc.tile_pool(name="xg", bufs=2) as xgpool,
            tc.tile_pool(name="prep", bufs=2) as ppool,
            tc.tile_pool(name="prepbf", bufs=2) as pbfpool,
            tc.tile_pool(name="xgt", bufs=2) as xgtpool,
            tc.tile_pool(name="ostage", bufs=3) as opool,
            tc.tile_pool(name="pmain", bufs=4, space=PSUM) as pmain,
        ):
            # ---------------- constants / setup ----------------
            A_sb = cpool.tile([R, IN], BF)
            nc.sync.dma_start(A_sb[:], A[:])
            AT_sb = cpool.tile([128, KT, R], BF)
            nc.sync.dma_start(AT_sb[:], AT.ap().rearrange("(t p) r -> p t r", p=128))
            BT_sb = cpool.tile([R, SH], BF)
            nc.sync.dma_start(BT_sb[:], BTd[:])
            q_sb = cpool.tile([128, QB, SH], I32)
            nc.sync.dma_start(q_sb[:], qd.ap().rearrange("(b p) o -> p b o", p=128))
            sc_sb = cpool.tile([NG, SH], F32)
            nc.sync.dma_start(sc_sb[:], scd[:])
            qz_sb = cpool.tile([NG, SH // PF], I32)
            nc.sync.dma_start(qz_sb[:], qzd[:])
            gft_sb = cpool.tile([128, KT], F32)
            nc.sync.dma_start(gft_sb[:], gftd[:])
            iota_sb = cpool.tile([128, NG], F32)
            nc.sync.dma_start(iota_sb[:], iotad[:])
            c32_sb = cpool.tile([NG, 1], F32)
            nc.sync.dma_start(c32_sb[:], c32d[:])
            gb_sb = cpool.tile([NG, IN], F32)
            nc.sync.dma_start(gb_sb[:], gbd[:])
            sum4_sb = cpool.tile([128, NG], BF)
            nc.sync.dma_start(sum4_sb[:], sum4d[:])

            # scales in bf16 (rhs of the T1 gather matmul)
            sc_bf = cpool.tile([NG, SH], BF)
            nc.vector.tensor_copy(sc_bf[:], sc_sb[:])
            sc192 = cpool.tile([NG, SH], F32)
            nc.vector.tensor_scalar(sc192[:], sc_sb[:], 1.0 / 192.0, None,
                                    AOP.mult)

            # zeros unpack: [NG, SH//PF] int32 -> [NG, SH] (int, then cast)
            zeros_i = cpool.tile([NG, SH], I32)
            for j in range(PF):
                nc.vector.tensor_scalar(
                    zeros_i[:, j::PF], qz_sb[:], 4 * j, MAXQ,
                    AOP.logical_shift_right, AOP.bitwise_and)
            zeros_f = cpool.tile([NG, SH], F32)
            nc.vector.tensor_copy(zeros_f[:], zeros_i[:])

            # one-hot tiles: G[p, t, g] = (g_idx[128t+p] == g)   (bf16)
            G_sb = cpool.tile([128, KT, NG], BF)
            for t in range(KT):
                nc.vector.tensor_scalar(
                    G_sb[:, t, :], iota_sb[:], gft_sb[:, t:t + 1], None,
                    AOP.is_equal)
            # transposed one-hot: GT[g, t, i] = (g_idx[128t+i] == g)
            GT_sb = cpool.tile([NG, KT, 128], BF)
            for t in range(KT):
                nc.gpsimd.tensor_scalar(
                    GT_sb[:, t, :], gb_sb[:, 128 * t:128 * (t + 1)],
                    c32_sb[:], None, AOP.is_equal)

            # A_grpT[r, g] = sum_{i in group g} A^T[i, r]   (exact ints)
            with tc.tile_pool(name="pmisc", bufs=1, space=PSUM) as pmisc:
                pAg = pmisc.tile([R, NG], F32)
                for t in range(KT):
                    nc.tensor.matmul(pAg[:], AT_sb[:, t, :], G_sb[:, t, :],
                                     start=(t == 0), stop=(t == KT - 1))
                AgT_sb = cpool.tile([R, NG], BF)
                nc.vector.tensor_copy(AgT_sb[:], pAg[:])
                # sumAB_g[g, o] = sum_{i in g} AB[i, o]  (exact ints)
                psAB = pmisc.tile([NG, SH], F32)
                nc.tensor.matmul(psAB[:], AgT_sb[:], BT_sb[:],
                                 start=True, stop=True)
                sAB_sb = cpool.tile([NG, SH], F32)
                nc.vector.tensor_copy(sAB_sb[:], psAB[:])

            # x chunk for mc 0 (consumed by matmuls interleaved into prep)
            xg0 = xgpool.tile([128, KT, 512], BF, tag="xg")
            nc.sync.dma_start(xg0[:], xT_t[:, :, 0:512])

            pm0 = [pmain.tile([128, 512], F32, tag="pm", name=f"pm0_{i}")
                   for i in range(4)]

            # ---------------- prep loop over 32 k-tiles ----------------
            W_sb = []
            with (
                tc.tile_pool(name="pmg", bufs=1, space=PSUM) as pmgpool,
                tc.tile_pool(name="pab", bufs=1, space=PSUM) as pabpool,
                tc.tile_pool(name="pt1", bufs=1, space=PSUM) as pt1pool,
            ):
                pmg = pmgpool.tile([NG, SH], F32)   # group-sums of markers
                for t in range(KT):
                    pab = pabpool.tile([128, SH], F32)
                    nc.tensor.matmul(pab[:], A_sb[:, 128 * t:128 * (t + 1)],
                                     BT_sb[:], start=True, stop=True)
                    # w = (q >> 4j) & 15  (int32), then cast to f32
                    wi = ppool.tile([128, SH], I32, tag="wi")
                    nc.vector.tensor_scalar(wi[:], q_sb[:, t % QB, :],
                                            4 * (t // QB), MAXQ,
                                            AOP.logical_shift_right,
                                            AOP.bitwise_and)
                    wf = ppool.tile([128, SH], F32, tag="wf")
                    nc.gpsimd.tensor_copy(wf[:], wi[:])
                    wne15 = ppool.tile([128, SH], F32, tag="wne15")
                    nc.gpsimd.tensor_scalar(wne15[:], wf[:], float(MAXQ), None,
                                            AOP.is_lt)
                    wne0 = ppool.tile([128, SH], F32, tag="wne0")
                    nc.gpsimd.tensor_scalar(wne0[:], wf[:], 0.0, None,
                                            AOP.is_gt)
                    pp = ppool.tile([128, SH], F32, tag="pp")
                    nc.vector.scalar_tensor_tensor(pp[:], pab[:], 2.0,
                                                   wne15[:], AOP.is_ge,
                                                   AOP.mult)
                    nn = ppool.tile([128, SH], F32, tag="nn")
                    nc.vector.scalar_tensor_tensor(nn[:], pab[:], -2.0,
                                                   wne0[:], AOP.is_le,
                                                   AOP.mult)
                    mf = ppool.tile([128, SH], F32, tag="mf")
                    nc.vector.tensor_sub(mf[:], pp[:], nn[:])
                    mbf = pbfpool.tile([128, SH], BF, tag="mbf")
                    nc.scalar.copy(mbf[:], mf[:])
                    # group-sums of markers (accumulated over all k-tiles)
                    nc.tensor.matmul(pmg[:], G_sb[:, t, :], mbf[:],
                                     start=(t == 0), stop=(t == KT - 1))
                    # T1 gather: scales[g_idx[i], o]
                    pt1 = pt1pool.tile([128, SH], F32)
                    nc.tensor.matmul(pt1[:], GT_sb[:, t, :], sc_bf[:],
                                     start=True, stop=True)
                    wpm = ppool.tile([128, SH], F32, tag="wpm")
                    nc.vector.tensor_add(wpm[:], wf[:], mf[:])
                    wt = wpool.tile([128, SH], BF, tag="w")
                    nc.vector.tensor_mul(wt[:], pt1[:], wpm[:])
                    W_sb.append(wt)
                    # keep PE busy: main matmuls for m-chunk 0, k-tile t
                    for mt in range(4 if mode != "nomain" else 0):
                        nc.tensor.matmul(
                            pm0[mt][:], xg0[:, t, 128 * mt:128 * (mt + 1)],
                            wt[:], start=(t == 0), stop=False)

                # ------------- T2 table -------------
                # sumAB2_g = sumAB_g - 1.5*sum_markers_g
                # T2 = sc/192*sumAB2_g - sc*zeros
                g1 = ppool.tile([NG, SH], F32, tag="pp")
                nc.vector.scalar_tensor_tensor(g1[:], pmg[:], -1.5, sAB_sb[:],
                                               AOP.mult, AOP.add)
                d1 = ppool.tile([NG, SH], F32, tag="nn")
                nc.vector.scalar_tensor_tensor(d1[:], zeros_f[:], -192.0,
                                               g1[:], AOP.mult, AOP.add)
                T2_sb = cpool.tile([NG, SH], BF)
                nc.vector.tensor_mul(T2_sb[:], d1[:], sc192[:])

            # ---------------- main loop over m-chunks ----------------
            with tc.tile_pool(name="pxg", bufs=4, space=PSUM) as pxgpool:
                for mc in range(MC if mode != "nomain" else 0):
                    if mc == 0:
                        xg = xg0
                    else:
                        xg = xgpool.tile([128, KT, 512], BF, tag="xg", name="xg")
                        nc.sync.dma_start(
                            xg[:], xT_t[:, :, 512 * mc:512 * (mc + 1)])
                    do_xgt = mode != "noxgt"
                    if do_xgt:
                        # xG^T chunk: col-packed one-hot matmuls (concurrent
                        # in the PE array via tile_position; 1 bank/group)
                        pxg = [pxgpool.tile([128, 512], F32, tag="pxg",
                                            name=f"pxg{c}") for c in range(4)]
                        for t in range(KT):
                            c = t % 4
                            nc.tensor.matmul(
                                pxg[c][32 * c:32 * (c + 1), :], G_sb[:, t, :],
                                xg[:, t, :], start=(t < 4),
                                stop=(t >= KT - 4),
                                tile_position=(0, 32 * c))
                        xg4 = xgtpool.tile([128, 512], BF, tag="xg4")
                        for c in range(4):
                            nc.vector.tensor_copy(
                                xg4[32 * c:32 * (c + 1), :],
                                pxg[c][32 * c:32 * (c + 1), :])
                        pfix = pxgpool.tile([NG, 512], F32, tag="pxg",
                                            name="pfix")
                        nc.tensor.matmul(pfix[:], sum4_sb[:], xg4[:],
                                         start=True, stop=True)
                        xgt = xgtpool.tile([NG, 512], BF, tag="xgt")
                        nc.vector.tensor_copy(xgt[:], pfix[:])

                    for mt in range(4):
                        if mc == 0:
                            pm = pm0[mt]
                            if do_xgt:
                                nc.tensor.matmul(
                                    pm[:], xgt[:, 128 * mt:128 * (mt + 1)],
                                    T2_sb[:], start=False, stop=True)
                            else:
                                nc.tensor.matmul(
                                    pm[:], xg[:, 0, 128 * mt:128 * (mt + 1)],
                                    W_sb[0][:], start=False, stop=True)
                        else:
                            pm = pmain.tile([128, 512], F32, tag="pm", name="pm")
                            if do_xgt:
                                nc.tensor.matmul(
                                    pm[:], xgt[:, 128 * mt:128 * (mt + 1)],
                                    T2_sb[:], start=True, stop=False)
                            for t in range(KT):
                                nc.tensor.matmul(
                                    pm[:], xg[:, t, 128 * mt:128 * (mt + 1)],
                                    W_sb[t][:], start=(t == 0 and not do_xgt),
                                    stop=(t == KT - 1))
                        ost = opool.tile([128, SH], F32)
                        nc.vector.tensor_copy(ost[:], pm[:])
                        nc.sync.dma_start(
                            outd[512 * mc + 128 * mt:
                                 512 * mc + 128 * (mt + 1), :], ost[:])
    nc.compile()
    return nc


def host_prep(x, lora_A, lora_B, qweight, qzeros, scales, g_idx):
    """Layout-only host prep: shard, permute IN axis, transpose, bf16-cast."""
    m_tot = int(np.prod(x.shape[:-1]))
    orig = (np.arange(IN) % (IN // PF)) * PF + np.arange(IN) // (IN // PF)

    X2 = np.asarray(x, np.float32).reshape(m_tot, IN)
    xT = np.ascontiguousarray(X2.T[orig]).astype(BF16)          # [IN, m_tot]
    Ap = np.ascontiguousarray(
        np.asarray(lora_A, np.float32)[:, orig]).astype(BF16)   # [R, IN]
    ATp = np.ascontiguousarray(Ap.T)                            # [IN, R]
    gperm = np.asarray(g_idx)[orig].astype(np.float32)          # [IN]
    gft = np.ascontiguousarray(gperm.reshape(KT, 128).T)        # [128, KT]
    gb = np.ascontiguousarray(np.broadcast_to(gperm, (NG, IN)))
    iota = np.ascontiguousarray(np.broadcast_to(
        np.arange(NG, dtype=np.float32), (128, NG)))
    c32 = np.arange(NG, dtype=np.float32).reshape(NG, 1)
    sum4 = (np.arange(128)[:, None] % NG
            == np.arange(NG)[None, :]).astype(BF16)

    in_maps = []
    for r in range(NCORES):
        cs = slice(SH * r, SH * (r + 1))
        zs = slice((SH // PF) * r, (SH // PF) * (r + 1))
        in_maps.append(dict(
            xT=xT, A=Ap, AT=ATp,
            BT=np.ascontiguousarray(
                np.asarray(lora_B, np.float32)[cs, :].T).astype(BF16),
            q=np.ascontiguousarray(np.asarray(qweight)[:, cs]),
            sc=np.ascontiguousarray(np.asarray(scales, np.float32)[:, cs]),
            qz=np.ascontiguousarray(np.asarray(qzeros)[:, zs]),
            gft=gft, iota=iota, c32=c32, gb=gb, sum4=sum4,
        ))
    return in_maps, m_tot


LAST_RESULT = None


def kernel(**inputs):
    global LAST_RESULT
    x = np.asarray(inputs["x"])
    in_maps, m_tot = host_prep(
        x, inputs["lora_A"], inputs["lora_B"], inputs["qweight"],
        inputs["qzeros"], inputs["scales"], inputs["g_idx"])
    nc = build_bass(m_tot)
    res = run_bass_kernel_spmd(nc, in_maps, core_ids=list(range(NCORES)))
    LAST_RESULT = res
    outs = [np.asarray(res.results[r]["out"], np.float32)
            for r in range(NCORES)]
    out = np.concatenate(outs, axis=1)
    return out.reshape(*x.shape[:-1], OUT)
